# revision 2
# baseline (speedup 1.0000x reference)
"""AuthorGroupAttention Trainium2 kernel, v2.

Data-parallel over batch: 8 samples -> 8 NeuronCores. Per-sample routing
resolved on host (combined [gen|rdr] Q/K weights per reader group).

v2 vs v1: fp8e4m3 DoubleRow matmuls wherever precision allows.
  - QK/V/O projections: hi/lo fp8 split, x@w ~= (xh+xl)@wh + xh@wl
    (drops xl@wl, ~1e-3 rel). DoubleRow contracts 256/instr at 0.5
    cyc/col -> 6/8 the f32r PE time.
  - reader scores + reader attention: straight fp8 DoubleRow (the reader
    path carries 10% of the probability weight; 3.2e-3 end-to-end).
  - generic scores: f32r; generic attention: bf16 exp x bf16 v.
  - exp on ScalarE, one [128,1024] activation per s-pair per path
    (gen -> bf16, rdr -> fp8); this is the ~255us engine floor.
  - PSUM plan (8 banks): gen scores 2 + rdr scores 2 + attn 2 + proj 2.
  - GPSIMD cannot touch PSUM: all PSUM drains are DVE; Pool (gpsimd)
    takes the SBUF-only combine work (1/Z broadcast, weighted sum,
    comb hi/lo split).
"""

import os
import sys

for _p in ("/opt/trn_rl_repo",):
    if os.path.isdir(_p) and _p not in sys.path:
        sys.path.insert(0, _p)

import numpy as np
import ml_dtypes

import concourse.bass as bass
import concourse.mybir as mybir
from concourse import bacc
from concourse.tile import TileContext
from concourse.bass_utils import run_bass_kernel_spmd

B, T, E, H, G = 8, 1024, 1024, 16, 4
D = E // H  # 64
SCALING = float(D) ** -0.5
W_G = 0.9 / 2.0
W_R = 0.1 / 2.0

F32 = mybir.dt.float32
F32R = mybir.dt.float32r
BF16 = mybir.dt.bfloat16
FP8 = mybir.dt.float8e4
U8 = mybir.dt.uint8
U16 = mybir.dt.uint16
DR = mybir.MatmulPerfMode.DoubleRow
EXP = mybir.ActivationFunctionType.Exp

EO = E // 128  # 8
SO = T // 128  # 8
OO = E // 128  # 8
NP2 = 4  # e-pair blocks in the lo phase

ONE_BF16 = 0x3F80
ONE_FP8 = 0x38  # e4m3 1.0
E4 = ml_dtypes.float8_e4m3  # TRN variant: max +-240


def build_nc():
    nc = bacc.Bacc(name="author_group_attention_v2")

    hs8 = nc.dram_tensor("hs8", [EO, 128, 2, T], FP8, kind="ExternalInput")
    wqA = nc.dram_tensor("wqA", [H, EO, 128, 2, 128], FP8, kind="ExternalInput")
    wkA = nc.dram_tensor("wkA", [H, EO, 128, 2, 128], FP8, kind="ExternalInput")
    wqB = nc.dram_tensor("wqB", [H, NP2, 128, 2, 128], FP8, kind="ExternalInput")
    wkB = nc.dram_tensor("wkB", [H, NP2, 128, 2, 128], FP8, kind="ExternalInput")
    wvA = nc.dram_tensor("wvA", [EO, 128, 2, E], FP8, kind="ExternalInput")
    wvB = nc.dram_tensor("wvB", [NP2, 128, 2, E], FP8, kind="ExternalInput")
    woA = nc.dram_tensor("woA", [OO, EO, 128, 2, 128], FP8, kind="ExternalInput")
    woB = nc.dram_tensor("woB", [OO, NP2, 128, 2, 128], FP8, kind="ExternalInput")
    bqk = nc.dram_tensor("bqk", [128, 2 * H], F32, kind="ExternalInput")
    wcol = nc.dram_tensor("wcol", [128, 4], F32, kind="ExternalInput")
    bo = nc.dram_tensor("bo", [128, OO], F32, kind="ExternalInput")
    outT = nc.dram_tensor("outT", [E, T], F32, kind="ExternalOutput")

    with TileContext(nc) as tc:
        from contextlib import ExitStack

        with ExitStack() as stack:
            const = stack.enter_context(tc.tile_pool(name="const", bufs=1))

            hs8_sb = const.tile([128, EO, 2, T], FP8, tag="hs8")
            # gen v blocks: [s_p, so, pair, 192] = [v_e |1|0*31|1|0*31| v_o]
            v_gen = const.tile([128, SO, H // 2, 192], BF16, tag="vgen")
            # rdr v blocks: [s_p, so2, j, h, 128]; even h: v 0:64, one at 64;
            # odd h: one at 32, v 64:128 (PSUM partition alignment).
            v8 = const.tile([128, SO // 2, 2, H, 128], FP8, tag="v8")
            comb = const.tile([128, EO, T], BF16, tag="comb")
            comb8 = const.tile([128, EO, 2, T], FP8, tag="comb8")
            bqk_sb = const.tile([128, 2 * H], F32, tag="bqk")
            wcol_sb = const.tile([128, 4], F32, tag="wcol")
            bo_sb = const.tile([128, OO], F32, tag="bo")

            nc.vector.memset(v_gen[:].bitcast(U16), 0)
            nc.vector.memset(v_gen[:, :, :, D].bitcast(U16), ONE_BF16)
            nc.vector.memset(v_gen[:, :, :, 96].bitcast(U16), ONE_BF16)
            nc.vector.memset(v8[:].bitcast(U8), 0)
            nc.vector.memset(v8[:, :, :, 0::2, 64].bitcast(U8), ONE_FP8)
            nc.vector.memset(v8[:, :, :, 1::2, 32].bitcast(U8), ONE_FP8)

            nc.sync.dma_start(bqk_sb[:], bqk[:])
            nc.sync.dma_start(wcol_sb[:], wcol[:])
            nc.sync.dma_start(bo_sb[:], bo[:])

            wpool = stack.enter_context(tc.tile_pool(name="wqk", bufs=3))
            qkpool = stack.enter_context(tc.tile_pool(name="qk", bufs=2))
            ppsum = stack.enter_context(
                tc.tile_pool(name="ppsum", bufs=1, space="PSUM")
            )

            def proj_steps(h, which):
                """Combined Q or K proj for head h: 24 DoubleRow matmuls
                (A: hi against (hsh+hsl) pairs, 8 eo x 2 nh; B: lo against
                hsh e-pairs, 4 x 2 nh), then gen drain (DVE, +bias, f32r
                rows 0:64), rdr drain (DVE, +bias, fp8 rows 64:128), then
                the cross-partition DMA shuffle to [32, 2, T]. None entries
                pace the pump so psum-reuse stalls don't head-of-line block
                the PE queue."""
                srcA = wqA if which == "q" else wkA
                srcB = wqB if which == "q" else wkB
                wtA = wpool.tile([128, EO, 2, 128], FP8, tag="wA")
                wtB = wpool.tile([128, NP2, 2, 128], FP8, tag="wB")
                nc.sync.dma_start(wtA[:], srcA[h].rearrange("e p j m -> p e j m"))
                nc.sync.dma_start(wtB[:], srcB[h].rearrange("e p j m -> p e j m"))
                dst_g = qkpool.tile([64, T], F32R, tag=which + "g")
                tmp8 = qkpool.tile([128, T], FP8, tag=which + "t8")
                dst_r = qkpool.tile([32, 2, T], FP8, tag=which + "r8")
                bias_col = 2 * h if which == "q" else 2 * h + 1
                state = {}
                steps = []

                def mk_mm(kind, idx, nh, first, last):
                    def step():
                        if first:
                            state[0] = ppsum.tile(
                                [128, T], F32, tag="proj", name="pq"
                            )
                        tsl = slice(nh * 512, (nh + 1) * 512)
                        if kind == "A":
                            lhsT = wtA[:, idx, :, :]
                            rhs = hs8_sb[:, idx, :, tsl]
                        else:
                            lhsT = wtB[:, idx, :, :]
                            rhs = hs8_sb[:, 2 * idx : 2 * idx + 2, 0, tsl]
                        nc.tensor.matmul(
                            state[0][:, tsl],
                            lhsT,
                            rhs,
                            start=(kind == "A" and idx == 0),
                            stop=last,
                            perf_mode=DR,
                        )
                    return step

                for nh in range(2):
                    for eo in range(EO):
                        steps.append(
                            mk_mm("A", eo, nh, nh == 0 and eo == 0, False)
                        )
                    for m2 in range(NP2):
                        steps.append(mk_mm("B", m2, nh, False, m2 == NP2 - 1))

                def drain_gen():
                    nc.vector.tensor_scalar_add(
                        dst_g[:, :],
                        state[0][0:64, :],
                        bqk_sb[0:64, bias_col : bias_col + 1],
                    )

                def drain_rdr():
                    nc.vector.tensor_scalar_add(
                        tmp8[64:128, :],
                        state[0][64:128, :],
                        bqk_sb[64:128, bias_col : bias_col + 1],
                    )

                def shuffle0():
                    # d = 32j + p: dst[p, j, :] = src[64 + 32j + p, :]
                    nc.sync.dma_start(dst_r[:, 0, :], tmp8[64:96, :])

                def shuffle1():
                    nc.sync.dma_start(dst_r[:, 1, :], tmp8[96:128, :])

                steps += [drain_gen, drain_rdr, shuffle0, shuffle1,
                          None, None, None]
                return dst_g, dst_r, steps

            def pump_n(pump, n):
                while n > 0 and pump:
                    s = pump.pop(0)
                    n -= 1
                    if s is not None:
                        s()

            # ---------------- v projection -------------------------------
            with tc.tile_pool(name="vpsum", bufs=3, space="PSUM") as vpsum, \
                 tc.tile_pool(name="wvp", bufs=1) as wvp:
                wvA_sb = wvp.tile([128, EO, 2, E], FP8, tag="wvA")
                wvB_sb = wvp.tile([128, NP2, 2, E], FP8, tag="wvB")
                hs8_r = hs8.rearrange("e p j t -> p e j t")
                wvA_r = wvA.rearrange("e p j o -> p e j o")
                wvB_r = wvB.rearrange("e p j o -> p e j o")
                for eo in range(EO):
                    nc.sync.dma_start(hs8_sb[:, eo], hs8_r[:, eo])
                    nc.sync.dma_start(wvA_sb[:, eo], wvA_r[:, eo])
                for m2 in range(NP2):
                    nc.sync.dma_start(wvB_sb[:, m2], wvB_r[:, m2])
                Qg, Qr, steps_q0 = proj_steps(0, "q")
                Kg, Kr, steps_k0 = proj_steps(0, "k")
                pump0 = steps_q0 + steps_k0
                for so in range(SO):
                    ssl = slice(so * 128, (so + 1) * 128)
                    pv = vpsum.tile([128, T], F32, tag="vproj", name=f"pv{so}")
                    for nh in range(2):
                        osl = slice(nh * 512, (nh + 1) * 512)
                        for eo in range(EO):
                            nc.tensor.matmul(
                                pv[:, osl],
                                hs8_sb[:, eo, :, ssl],
                                wvA_sb[:, eo, :, osl],
                                start=(eo == 0),
                                stop=False,
                                perf_mode=DR,
                            )
                        for m2 in range(NP2):
                            nc.tensor.matmul(
                                pv[:, osl],
                                hs8_sb[:, 2 * m2 : 2 * m2 + 2, 0, ssl],
                                wvB_sb[:, m2, :, osl],
                                start=False,
                                stop=(m2 == NP2 - 1),
                                perf_mode=DR,
                            )
                        pump_n(pump0, 2 if so >= 6 else 1)
                    # gen v fill (bf16)
                    pv4 = pv.rearrange("p (m two d) -> p m two d", two=2, d=D)
                    nc.vector.tensor_copy(v_gen[:, so, :, 0:D], pv4[:, :, 0, :])
                    nc.vector.tensor_copy(
                        v_gen[:, so, :, 128 : 128 + D], pv4[:, :, 1, :]
                    )
                    # rdr v8 fill (fp8): even h -> block cols 0:64,
                    # odd h -> block cols 64:128 (= 192 offset in 256-pairs)
                    v8f = v8[:, so // 2, so % 2, :, :].rearrange(
                        "p h m -> p (h m)"
                    ).rearrange("p (m2 c) -> p m2 c", c=256)
                    pvf = pv.rearrange("p (m2 c) -> p m2 c", c=128)
                    nc.vector.tensor_copy(v8f[:, :, 0:64], pvf[:, :, 0:64])
                    nc.vector.tensor_copy(
                        v8f[:, :, 192:256], pvf[:, :, 64:128]
                    )

                while pump0:
                    s = pump0.pop(0)
                    if s is not None:
                        s()

            # ---------------- attention main loop -------------------------
            with ExitStack() as attn_stack:
                expp = attn_stack.enter_context(tc.tile_pool(name="exg", bufs=3))
                ex8p = attn_stack.enter_context(tc.tile_pool(name="exr", bufs=3))
                rawp = attn_stack.enter_context(tc.tile_pool(name="raw", bufs=2))
                zp = attn_stack.enter_context(tc.tile_pool(name="z", bufs=1))
                bcp = attn_stack.enter_context(tc.tile_pool(name="bc", bufs=2))
                gpsum = attn_stack.enter_context(
                    tc.tile_pool(name="gpsum", bufs=1, space="PSUM")
                )
                rpsum = attn_stack.enter_context(
                    tc.tile_pool(name="rpsum", bufs=1, space="PSUM")
                )
                apsum = attn_stack.enter_context(
                    tc.tile_pool(name="apsum", bufs=1, space="PSUM")
                )

                def emit_attn(pend, pag, par_, h, voff):
                    exg, exr, k = pend
                    for i in range(2):
                        s = 2 * k + i
                        nc.tensor.matmul(
                            pag[:],
                            v_gen[:, s, h // 2, voff : voff + 128],
                            exg[:, i * 512 : (i + 1) * 512],
                            start=(s == 0),
                            stop=(s == SO - 1),
                        )
                    nc.tensor.matmul(
                        par_[:],
                        v8[:, k, :, h, :],
                        exr[:].rearrange("p (j t) -> p j t", j=2),
                        start=(k == 0),
                        stop=(k == SO // 2 - 1),
                        perf_mode=DR,
                    )

                for h in range(H):
                    par_odd = h % 2
                    abase = 64 * par_odd
                    zrow = 64 if par_odd == 0 else 32
                    voff = 64 * par_odd
                    zrec = zp.tile([128, 2 * T], BF16, tag="zrec")
                    pump = []
                    nxt = None
                    if h + 1 < H:
                        nQg, nQr, sq = proj_steps(h + 1, "q")
                        nKg, nKr, sk = proj_steps(h + 1, "k")
                        nxt = (nQg, nQr, nKg, nKr)
                        pump = sq + sk

                    for th in range(2):
                        tsl = slice(th * 512, (th + 1) * 512)
                        pag = apsum.tile([128, 512], F32, tag="ag")
                        par_ = apsum.tile([128, 512], F32, tag="ar")
                        pend = None
                        for k in range(SO // 2):
                            psg = gpsum.tile([128, T], F32, tag="sg")
                            psr = rpsum.tile([128, T], F32, tag="sr")
                            for i in range(2):
                                s = 2 * k + i
                                ssl = slice(s * 128, (s + 1) * 128)
                                isl = slice(i * 512, (i + 1) * 512)
                                nc.tensor.matmul(
                                    psg[:, isl], Kg[:, ssl], Qg[:, tsl],
                                    start=True, stop=True,
                                )
                                nc.tensor.matmul(
                                    psr[:, isl],
                                    Kr[:, :, ssl],
                                    Qr[:, :, tsl],
                                    start=True, stop=True, perf_mode=DR,
                                )
                            exg = expp.tile([128, T], BF16, tag="exg")
                            nc.scalar.activation(exg[:], psg[:], EXP,
                                                 scale=SCALING)
                            exr = ex8p.tile([128, T], FP8, tag="exr")
                            nc.scalar.activation(exr[:], psr[:], EXP,
                                                 scale=SCALING)
                            if pend is not None:
                                emit_attn(pend, pag, par_, h, voff)
                            pend = (exg, exr, k)
                            pump_n(pump, 4)
                        emit_attn(pend, pag, par_, h, voff)

                        # ---- drain + combine (v1 scheme, bf16) ----
                        rawg = rawp.tile([128, 512], BF16, tag="rg")
                        rawr = rawp.tile([128, 512], BF16, tag="rr")
                        if par_odd == 0:
                            nc.vector.tensor_scalar_mul(
                                rawg[0:65, :], pag[0:65, :], wcol_sb[0:65, 0:1]
                            )
                            nc.vector.tensor_scalar_mul(
                                rawr[0:65, :], par_[0:65, :], wcol_sb[0:65, 1:2]
                            )
                        else:
                            nc.vector.tensor_scalar_mul(
                                rawg[64:128, :], pag[64:128, :], W_G * W_G
                            )
                            nc.vector.tensor_scalar_mul(
                                rawg[32:33, :], pag[32:33, :], W_G
                            )
                            nc.vector.tensor_scalar_mul(
                                rawr[64:128, :], par_[64:128, :], W_R * W_R
                            )
                            nc.vector.tensor_scalar_mul(
                                rawr[32:33, :], par_[32:33, :], W_R
                            )
                        zsl = slice(zrow, zrow + 1)
                        czg = slice(th * 512, th * 512 + 512)
                        czr = slice(T + th * 512, T + th * 512 + 512)
                        with nc.allow_low_precision(reason="1/Z bf16"):
                            nc.vector.reciprocal(zrec[zsl, czg], rawg[zsl, :])
                            nc.vector.reciprocal(zrec[zsl, czr], rawr[zsl, :])
                        nc.sync.dma_start(zrec[0:1, czg], zrec[zsl, czg])
                        nc.sync.dma_start(zrec[0:1, czr], zrec[zsl, czr])
                        bcg = bcp.tile([128, 512], BF16, tag="bg")
                        bcr = bcp.tile([128, 512], BF16, tag="br")
                        nc.gpsimd.partition_broadcast(bcg[:], zrec[0:1, czg])
                        nc.gpsimd.partition_broadcast(bcr[:], zrec[0:1, czr])
                        asl2 = slice(abase, abase + 64)
                        nc.gpsimd.tensor_mul(
                            rawg[asl2, :], rawg[asl2, :], bcg[asl2, :]
                        )
                        nc.gpsimd.tensor_mul(
                            rawr[asl2, :], rawr[asl2, :], bcr[asl2, :]
                        )
                        nc.gpsimd.tensor_add(
                            comb[asl2, h // 2, tsl], rawg[asl2, :],
                            rawr[asl2, :],
                        )
                        if par_odd == 1 and th == 1:
                            eo = h // 2
                            nc.gpsimd.tensor_copy(comb8[:, eo, 0, :],
                                                  comb[:, eo, :])
                            nc.gpsimd.tensor_sub(
                                comb8[:, eo, 1, :], comb[:, eo, :],
                                comb8[:, eo, 0, :],
                            )
                    while pump:
                        s = pump.pop(0)
                        if s is not None:
                            s()
                    if nxt is not None:
                        Qg, Qr, Kg, Kr = nxt

            # ---------------- output projection ---------------------------
            with tc.tile_pool(name="tail", bufs=3) as tailp, tc.tile_pool(
                name="outsb", bufs=2
            ) as outp, tc.tile_pool(name="opsum", bufs=2, space="PSUM") as opsum:
                for j in range(OO):
                    wtA = tailp.tile([128, EO, 2, 128], FP8, tag="woA")
                    wtB = tailp.tile([128, NP2, 2, 128], FP8, tag="woB")
                    nc.sync.dma_start(
                        wtA[:], woA[j].rearrange("e p j2 m -> p e j2 m")
                    )
                    nc.sync.dma_start(
                        wtB[:], woB[j].rearrange("e p j2 m -> p e j2 m")
                    )
                    po = opsum.tile([128, T], F32, tag="oproj")
                    ot = outp.tile([128, T], F32, tag="ot")
                    for nh in range(2):
                        tsl = slice(nh * 512, (nh + 1) * 512)
                        for eo in range(EO):
                            nc.tensor.matmul(
                                po[:, tsl],
                                wtA[:, eo, :, :],
                                comb8[:, eo, :, tsl],
                                start=(eo == 0),
                                stop=False,
                                perf_mode=DR,
                            )
                        for m2 in range(NP2):
                            nc.tensor.matmul(
                                po[:, tsl],
                                wtB[:, m2, :, :],
                                comb8[:, 2 * m2 : 2 * m2 + 2, 0, tsl],
                                start=False,
                                stop=(m2 == NP2 - 1),
                                perf_mode=DR,
                            )
                        nc.vector.tensor_scalar_add(
                            ot[:, tsl], po[:, tsl], bo_sb[:, j : j + 1]
                        )
                        nc.sync.dma_start(
                            outT[j * 128 : (j + 1) * 128, tsl], ot[:, tsl]
                        )

    nc.finalize()
    return nc


_NC_CACHE = {}


def get_nc():
    if "nc" not in _NC_CACHE:
        _NC_CACHE["nc"] = build_nc()
    return _NC_CACHE["nc"]


def _split8(x):
    """fp8e4m3 hi/lo split: x ~= hi + lo with |err| ~ 1e-3 |x|."""
    hi = np.clip(x, -240, 240).astype(E4)
    lo = np.clip(x - hi.astype(np.float32), -240, 240).astype(E4)
    return hi, lo


def _qk_ab(wcomb):
    """wcomb [E, H, 128] f32 -> (A [H, EO, 128, 2, 128], B [H, NP2, 128, 2, 128])."""
    hi, lo = _split8(wcomb)
    # A: hi, e-tiles on axis, duplicated on j
    a = hi.reshape(EO, 128, H, 128).transpose(2, 0, 1, 3)  # [H, EO, 128, 128]
    A = np.ascontiguousarray(
        np.broadcast_to(a[:, :, :, None, :], (H, EO, 128, 2, 128))
    )
    # B: lo, e-pairs on j
    b = lo.reshape(NP2, 2, 128, H, 128).transpose(3, 0, 2, 1, 4)
    Bt = np.ascontiguousarray(b)  # [H, NP2, 128, 2, 128]
    return A, Bt


def _mat_ab(w):
    """w [E, O] f32 -> (A [EO, 128, 2, O] dup-j hi, B [NP2, 128, 2, O] e-pair lo)."""
    hi, lo = _split8(w)
    O = w.shape[1]
    a = hi.reshape(EO, 128, O)
    A = np.ascontiguousarray(np.broadcast_to(a[:, :, None, :], (EO, 128, 2, O)))
    Bt = np.ascontiguousarray(lo.reshape(NP2, 2, 128, O).transpose(0, 2, 1, 3))
    return A, Bt


def _host_prep(hidden_states, reader_token, Wq, bq, Wk, bk, Wv, bv, Wo, bo,
               RWq, Rbq, RWk, Rbk, RWv, Rbv):
    f = np.float32
    hs = np.asarray(hidden_states, f)
    tok = np.asarray(reader_token).astype(np.int64)
    WqT = np.ascontiguousarray(np.asarray(Wq, f).T)  # [e, o]
    WkT = np.ascontiguousarray(np.asarray(Wk, f).T)
    WvT = np.ascontiguousarray(np.asarray(Wv, f).T)
    WoT = np.ascontiguousarray(np.asarray(Wo, f).T)
    RWqT = np.transpose(np.asarray(RWq, f), (0, 2, 1))
    RWkT = np.transpose(np.asarray(RWk, f), (0, 2, 1))
    bq = np.asarray(bq, f); bk = np.asarray(bk, f)
    bv = np.asarray(bv, f); bo_ = np.asarray(bo, f)
    Rbq = np.asarray(Rbq, f); Rbk = np.asarray(Rbk, f)

    # v-bias folds into output bias (probs rows sum to exactly 0.5)
    bo_eff = bo_ + 0.5 * (np.asarray(Wo, f) @ bv)
    bo_t = np.ascontiguousarray(bo_eff.reshape(OO, 128).T)

    WqT_h = WqT.reshape(E, H, D)
    WkT_h = WkT.reshape(E, H, D)

    wcol_t = np.zeros((128, 4), f)
    wcol_t[0:64, 0] = W_G * W_G
    wcol_t[64, 0] = W_G
    wcol_t[0:64, 1] = W_R * W_R
    wcol_t[64, 1] = W_R
    wcol_t[64:128, 2] = W_G * W_G
    wcol_t[32, 2] = W_G
    wcol_t[64:128, 3] = W_R * W_R
    wcol_t[32, 3] = W_R

    wvA_np, wvB_np = _mat_ab(WvT)
    # wo laid out per output j-tile: [OO, EO, 128, 2, 128]
    woA_f, woB_f = _mat_ab(WoT)  # [EO, 128, 2, E], [NP2, 128, 2, E]
    woA_np = np.ascontiguousarray(
        woA_f.reshape(EO, 128, 2, OO, 128).transpose(3, 0, 1, 2, 4)
    )
    woB_np = np.ascontiguousarray(
        woB_f.reshape(NP2, 128, 2, OO, 128).transpose(3, 0, 1, 2, 4)
    )

    in_maps = []
    percore = {}
    for b in range(B):
        g = int(tok[b])
        if g not in percore:
            wqc = np.empty((E, H, 128), f)
            wqc[:, :, :D] = WqT_h
            wqc[:, :, D:] = RWqT[g].reshape(E, H, D)
            wkc = np.empty((E, H, 128), f)
            wkc[:, :, :D] = WkT_h
            wkc[:, :, D:] = RWkT[g].reshape(E, H, D)
            bqk_t = np.empty((128, 2 * H), f)
            bqk_t[:D, 0::2] = bq.reshape(H, D).T
            bqk_t[D:, 0::2] = Rbq[g].reshape(H, D).T
            bqk_t[:D, 1::2] = bk.reshape(H, D).T
            bqk_t[D:, 1::2] = Rbk[g].reshape(H, D).T
            qA, qB = _qk_ab(wqc)
            kA, kB = _qk_ab(wkc)
            percore[g] = (qA, qB, kA, kB, bqk_t)
        qA, qB, kA, kB, bqk_t = percore[g]
        hsT = hs[b].T  # [E, T]
        hi, lo = _split8(hsT)
        hs8_np = np.empty((EO, 128, 2, T), E4)
        hs8_np[:, :, 0, :] = hi.reshape(EO, 128, T)
        hs8_np[:, :, 1, :] = lo.reshape(EO, 128, T)
        in_maps.append(
            {
                "hs8": hs8_np,
                "wqA": qA, "wqB": qB, "wkA": kA, "wkB": kB,
                "wvA": wvA_np, "wvB": wvB_np,
                "woA": woA_np, "woB": woB_np,
                "bqk": bqk_t,
                "wcol": wcol_t,
                "bo": bo_t,
            }
        )
    return in_maps


def kernel(**inputs) -> np.ndarray:
    in_maps = _host_prep(**inputs)
    nc = get_nc()
    res = run_bass_kernel_spmd(nc, in_maps, list(range(B)))
    out = np.stack([res.results[c]["outT"].T for c in range(B)], axis=0)
    return np.ascontiguousarray(out.astype(np.float32))
